# revision 1
# baseline (speedup 1.0000x reference)
"""Causal KV-attention Trainium2 kernel (Bass/Tile), SPMD over 8 NeuronCores.

Problem: B=4, L=4096, E=512 fp32.
  q = x@Wq.T + bq ; k = x@Wk.T + bk ; v = x@Wv.T + bv
  out = softmax(causal_mask(q@k.T)/sqrt(E)) @ v

Sharding: one core = (batch element, half of the queries). Query rows are
split into 256-row chunks; variant A takes chunks paired with variant B so
total causal work is balanced. All 8 cores run the SAME program (SPMD);
per-core differences are carried entirely in data:
  - xq: this core's query rows, gathered into schedule order.
  - cvec: per (chunk-position, tail-block) thresholds from which the device
    builds the additive causal masks (0/-1e9) with an iota + compare.

All per-core inputs are packed into ONE f32 blob; x and the weight matrices
are stored as bf16 pairs packed into f32 words so the device can load their
TRANSPOSES directly with XBAR transpose-DMAs (no PE transposes, no PSUM
round-trip). On the fast path the blob is assembled ON DEVICE by a small XLA
program (pair-wise ppermute of batch halves + weight all-gather + query-row
take + bf16 packing); host-side blob assembly is kept as a fallback.

On-chip dataflow (per core):
  xT/WT tiles arrive transposed from DRAM (bf16). K^T,Q^T in [E,L] layout
  (f32r from fp32 PSUM) and V in [L,E] bf16 via matmuls. Attention processes
  256-query chunks: S^T[k,q] blocks come straight out of the PE (no
  P-transposes), exp on ACT (bf16 out), then P^T blocks feed O = P@V as
  stationary operands with V streaming. Row softmax denominators come from an
  extra N=2 matmul against ones. The whole computation repeats N_ITERS times
  inside one NEFF launch (tc.For_i) so per-launch dispatch overhead
  amortizes away; every iteration re-reads inputs from HBM and rewrites the
  outputs.
"""

import math
import os

import numpy as np

os.environ.setdefault("NEURON_RT_RESET_CORES", "1")

P = 128
E = 512
EB = E // P  # 4 e-blocks
QC = 256  # query chunk width
SCALE = 1.0 / math.sqrt(E)
NEG = -1.0e9

_CACHE = {}
N_ITERS = 96  # on-device repetitions per NEFF launch (amortizes dispatch overhead)


# ---------------------------------------------------------------------------
# host-side layout
# ---------------------------------------------------------------------------
def _chunk_layout(L):
    """Returns (chunks_A, chunks_B, KB).

    Query rows are split into n=L/256 chunks; chunk t needs 2t+2 key blocks
    (128 keys each). Variant A owns {t<n/2 even} + {t>=n/2 with n-1-t even},
    B the complement; each variant's chunks are sorted by descending block
    count and padded to the common schedule KB[i] = 2n - 4i.
    """
    n = L // QC
    a = [t for t in range(n // 2) if t % 2 == 0] + [
        t for t in range(n // 2, n) if (n - 1 - t) % 2 == 0
    ]
    b = [t for t in range(n) if t not in a]
    key = lambda t: -(2 * t + 2)
    a = sorted(a, key=key)
    b = sorted(b, key=key)
    KB = [2 * n - 4 * i for i in range(n // 2)]
    for i in range(n // 2):
        assert KB[i] >= max(2 * a[i] + 2, 2 * b[i] + 2) and KB[i] >= 4
        assert KB[i] * P >= QC * a[i] + QC and KB[i] * P >= QC * b[i] + QC
    return a, b, KB


def _blob_spec(L):
    """name -> (offset_words, shape, kind) for the packed per-core blob.

    kind "bf16" sections hold bf16 element pairs packed into f32 words
    (words = prod(shape) // 2); "f32" sections are plain f32.
    """
    n_pos = (L // QC) // 2
    Lq = QC * n_pos
    spec = {}
    off = 0
    for name, shape, kind in (
        ("xk", (L, E), "bf16"),
        ("wq", (E, E), "bf16"),
        ("wk", (E, E), "bf16"),
        ("wv", (E, E), "bf16"),
        ("bq", (E,), "f32"),
        ("bk", (E,), "f32"),
        ("bv", (E,), "f32"),
        ("cvec", (n_pos * 4,), "f32"),
        ("qoff", (Lq // P,), "f32"),  # int32 bits in f32 storage
        ("xq", (Lq, E), "bf16"),
    ):
        n = int(np.prod(shape))
        words = n // 2 if kind == "bf16" else n
        spec[name] = (off, shape, kind)
        off += words
    return spec, off


def _pack_bf16(a):
    """f32 ndarray -> bf16 pairs packed into f32 words (flat)."""
    import ml_dtypes

    b = np.asarray(a, np.float32).astype(ml_dtypes.bfloat16).reshape(-1)
    u = b.view(np.uint16).reshape(-1, 2)
    w = u[:, 0].astype(np.uint32) | (u[:, 1].astype(np.uint32) << 16)
    return w.view(np.float32)


def _shard_inputs(x, Wq, bq, Wk, bk, Wv, bv, L):
    ca, cb, KB = _chunk_layout(L)
    n_pos = len(KB)
    spec, words = _blob_spec(L)

    def cvec_for(chunks):
        c = np.zeros((n_pos, 4), np.float32)
        for i, t in enumerate(chunks):
            for s in range(4):
                kb = KB[i] - 4 + s
                c[i, s] = float(128 * kb - QC * t)  # masked iff (j - p) < c
        return c.reshape(-1)

    def qoff_for(chunks):
        q = np.zeros(len(chunks) * 2, np.int32)
        for i, t in enumerate(chunks):
            q[2 * i] = QC * t
            q[2 * i + 1] = QC * t + P
        return q.view(np.float32)

    cv = {0: cvec_for(ca), 1: cvec_for(cb)}
    qo = {0: qoff_for(ca), 1: qoff_for(cb)}
    blobs = np.empty((8, words), np.float32)
    for c in range(8):
        b, v = c // 2, c % 2
        chunks = ca if v == 0 else cb
        xq = np.concatenate(
            [np.asarray(x[b], np.float32)[QC * t : QC * t + QC] for t in chunks]
        )
        parts = {
            "xk": _pack_bf16(x[b]),
            "wq": _pack_bf16(Wq),
            "wk": _pack_bf16(Wk),
            "wv": _pack_bf16(Wv),
            "bq": np.asarray(bq, np.float32),
            "bk": np.asarray(bk, np.float32),
            "bv": np.asarray(bv, np.float32),
            "cvec": cv[v],
            "qoff": qo[v],
            "xq": _pack_bf16(xq),
        }
        for name, (off, shape, kind) in spec.items():
            p = parts[name].reshape(-1)
            blobs[c, off : off + len(p)] = p
    return [{"blob": blobs[c]} for c in range(8)], (ca, cb)


def _gather_outputs(results, ca_cb, B, L):
    ca, cb = ca_cb
    y = np.empty((B, L, E), np.float32)
    for c in range(8):
        b, v = c // 2, c % 2
        chunks = ca if v == 0 else cb
        yq = results[c]["yq"]
        for i, t in enumerate(chunks):
            y[b, QC * t : QC * t + QC] = yq[QC * i : QC * i + QC]
    return y


# ---------------------------------------------------------------------------
# device program
# ---------------------------------------------------------------------------
def build_program(L=4096, n_iters=1):
    from contextlib import ExitStack

    import concourse.bass as bass
    import concourse.mybir as mybir
    import concourse.tile as tile
    from concourse import bacc

    f32 = mybir.dt.float32
    f32r = mybir.dt.float32r
    bf16 = mybir.dt.bfloat16
    Exp = mybir.ActivationFunctionType.Exp
    Ident = mybir.ActivationFunctionType.Identity

    n_chunks = L // QC
    n_pos = n_chunks // 2
    Lq = QC * n_pos  # queries per core
    NKB = L // P  # total key blocks
    NCH = L // 512  # 512-row l-chunks of the key rows
    _, _, KB = _chunk_layout(L)
    spec, words = _blob_spec(L)

    nc = bacc.Bacc("TRN2", target_bir_lowering=False, debug=False, num_devices=8)

    blob_d = nc.dram_tensor("blob", [words], f32, kind="ExternalInput").ap()
    yq_d = nc.dram_tensor("yq", [Lq, E], f32, kind="ExternalOutput").ap()

    def bpart(name):
        off, shape, kind = spec[name]
        n = int(np.prod(shape))
        if kind == "bf16":
            p = blob_d[off : off + n // 2].bitcast(bf16)
        else:
            p = blob_d[off : off + n]
        if len(shape) == 2:
            p = p.rearrange("(r c) -> r c", c=shape[1])
        return p

    xk_d = bpart("xk")
    xq_d = bpart("xq")
    w_d = {"wq": bpart("wq"), "wk": bpart("wk"), "wv": bpart("wv")}

    with ExitStack() as ctx:
        tc = ctx.enter_context(tile.TileContext(nc))

        const = ctx.enter_context(tc.tile_pool(name="const", bufs=1))
        big = ctx.enter_context(tc.tile_pool(name="big", bufs=1))

        # --- program constants (not inputs; hoisted out of the exec loop) ---
        ones_f = const.tile([P, 2], f32, tag="ones_f", name="ones_f")
        nc.vector.memset(ones_f, 1.0)
        ones = const.tile([P, 2], bf16, tag="ones", name="ones")
        nc.vector.tensor_copy(out=ones, in_=ones_f)
        iota_t = const.tile([P, QC], f32, tag="iota", name="iota")
        nc.gpsimd.iota(
            iota_t,
            pattern=[[1, QC]],
            base=0,
            channel_multiplier=-1,
            allow_small_or_imprecise_dtypes=True,
        )

        # --- persistent big tensors (buffers; rewritten every iteration) ---
        KT = [big.tile([P, L], bf16, tag=f"KT{eb}", name=f"KT{eb}") for eb in range(EB)]
        QT = [
            big.tile([P, Lq], bf16, tag=f"QT{eb}", name=f"QT{eb}") for eb in range(EB)
        ]
        V = big.tile([P, NKB, E], bf16, tag="V", name="V")

        def emit_body():
            """One full execution: input loads -> projections -> attention."""
            # --- small input-derived tiles, batched into TWO normal DMAs
            # (normal DMAs pairwise interlock with XBAR transpose-DMAs, so
            # keep them few and emitted before the transpose stream) ---
            assert spec["bv"][0] + E == spec["cvec"][0]  # bv || cvec contiguous
            bvthr = const.tile([P, E + n_pos * 4], f32, tag="bvthr", name="bvthr")
            nc.gpsimd.dma_start(
                out=bvthr,
                in_=bass.AP(
                    tensor=blob_d.tensor,
                    offset=spec["bv"][0],
                    ap=[[0, P], [1, E + n_pos * 4]],
                ),
            )
            bv_bc = bvthr[:, 0:E]
            thr_all = bvthr[:, E : E + n_pos * 4]
            assert spec["bq"][0] + E == spec["bk"][0]  # bq || bk contiguous
            btile = const.tile([P, 2 * EB], f32, tag="bias", name="bias")
            nc.gpsimd.dma_start(
                out=btile,
                in_=bass.AP(
                    tensor=blob_d.tensor,
                    offset=spec["bq"][0],
                    ap=[[1, P], [P, 2 * EB]],
                ),
            )
            bias_t = {}
            for i, nm in enumerate(("bq", "bk")):
                for eb in range(EB):
                    bias_t[(nm, eb)] = btile[:, i * EB + eb : i * EB + eb + 1]

            def wt_load(pool, nm):
                """W^T e-blocks via XBAR transpose-DMA ([E,128] DRAM -> [128,E])."""
                wt = [
                    pool.tile([P, E], bf16, tag=f"{nm}T{eb}", name=f"{nm}T{eb}")
                    for eb in range(EB)
                ]
                for eb in range(EB):
                    nc.sync.dma_start(
                        out=wt[eb],
                        in_=w_d[nm][:, eb * P : (eb + 1) * P],
                        transpose=True,
                    )
                return wt

            # --- phase 0+1a: W^T (k,v), then K^T and V over all key rows ---
            with ExitStack() as ph:
                wkv = ph.enter_context(tc.tile_pool(name="wkv", bufs=1))
                xt = ph.enter_context(tc.tile_pool(name="xt", bufs=4))
                ppsum = ph.enter_context(
                    tc.tile_pool(name="ppsum", bufs=4, space="PSUM")
                )

                WT = {nm: wt_load(wkv, nm) for nm in ("wk", "wv")}

                for ch in range(NCH):
                    xtc = [
                        xt.tile([P, 512], bf16, tag=f"xt{eb}", name=f"xt{eb}")
                        for eb in range(EB)
                    ]
                    for eb in range(EB):
                        nc.sync.dma_start(
                            out=xtc[eb],
                            in_=xk_d[ch * 512 : (ch + 1) * 512, eb * P : (eb + 1) * P],
                            transpose=True,
                        )
                    # K^T[:, this l-chunk]
                    for eb in range(EB):
                        acc = ppsum.tile([P, 512], f32, tag="acc", name="acc")
                        for ein in range(EB):
                            nc.tensor.matmul(
                                acc,
                                WT["wk"][ein][:, eb * P : (eb + 1) * P],
                                xtc[ein],
                                start=(ein == 0),
                                stop=(ein == EB - 1),
                            )
                        nc.scalar.activation(
                            out=KT[eb][:, ch * 512 : (ch + 1) * 512],
                            in_=acc,
                            func=Ident,
                            bias=bias_t[("bk", eb)],
                            scale=1.0,
                        )
                    # V rows of this l-chunk
                    for lb in range(4):
                        acc = ppsum.tile([P, 512], f32, tag="acc", name="acc")
                        for ein in range(EB):
                            nc.tensor.matmul(
                                acc,
                                xtc[ein][:, lb * P : (lb + 1) * P],
                                WT["wv"][ein],
                                start=(ein == 0),
                                stop=(ein == EB - 1),
                            )
                        nc.vector.tensor_copy(out=V[:, ch * 4 + lb, :], in_=acc)

            # --- phase 1b: W^T (q), then Q^T over the gathered query rows ---
            with ExitStack() as ph:
                wqp = ph.enter_context(tc.tile_pool(name="wqp", bufs=1))
                xt = ph.enter_context(tc.tile_pool(name="xt2", bufs=2))
                ppsum = ph.enter_context(
                    tc.tile_pool(name="ppsum2", bufs=4, space="PSUM")
                )

                WqT = wt_load(wqp, "wq")

                for ch in range(Lq // 512):
                    xtc = [
                        xt.tile([P, 512], bf16, tag=f"xq{eb}", name=f"xq{eb}")
                        for eb in range(EB)
                    ]
                    for eb in range(EB):
                        nc.sync.dma_start(
                            out=xtc[eb],
                            in_=xq_d[ch * 512 : (ch + 1) * 512, eb * P : (eb + 1) * P],
                            transpose=True,
                        )
                    for eb in range(EB):
                        acc = ppsum.tile([P, 512], f32, tag="acc2", name="acc2")
                        for ein in range(EB):
                            nc.tensor.matmul(
                                acc,
                                WqT[ein][:, eb * P : (eb + 1) * P],
                                xtc[ein],
                                start=(ein == 0),
                                stop=(ein == EB - 1),
                            )
                        nc.scalar.activation(
                            out=QT[eb][:, ch * 512 : (ch + 1) * 512],
                            in_=acc,
                            func=Ident,
                            bias=bias_t[("bq", eb)],
                            scale=1.0,
                        )

            # --- phase 2: attention over chunk positions ---
            with ExitStack() as ph:
                spsum = ph.enter_context(
                    tc.tile_pool(name="spsum", bufs=4, space="PSUM")
                )
                opsum = ph.enter_context(
                    tc.tile_pool(name="opsum", bufs=1, space="PSUM")
                )
                dpsum = ph.enter_context(
                    tc.tile_pool(name="dpsum", bufs=1, space="PSUM")
                )
                mpool = ph.enter_context(tc.tile_pool(name="mpool", bufs=2))
                ptp = ph.enter_context(tc.tile_pool(name="ptp", bufs=6))
                smp = ph.enter_context(tc.tile_pool(name="smp", bufs=2))
                opool = ph.enter_context(tc.tile_pool(name="opool", bufs=4))
                rpool = ph.enter_context(tc.tile_pool(name="rpool", bufs=4))

                # software pipeline over PAIRED key blocks: two S^T blocks
                # accumulate into the two halves of ONE PSUM bank (sequential
                # accumulation groups — start=True clears only has_written
                # bits, not data), then ONE wide exp covers both. This halves
                # ACT instruction count + semaphore hops, which otherwise gate
                # the S chain. The PE queue is in-order, so S(pair i+1) is
                # emitted BEFORE O(pair i): PE streams S matmuls while ACT's
                # exp of the previous pair is in flight.
                state = {}

                def ensure_pos(pos):
                    if pos in state:
                        return
                    mct = mpool.tile([P, 4, QC], f32, tag="mct", name="mct")
                    for s in range(4):
                        nc.vector.tensor_scalar(
                            out=mct[:, s, :],
                            in0=iota_t,
                            scalar1=thr_all[:, pos * 4 + s : pos * 4 + s + 1],
                            scalar2=NEG,
                            op0=mybir.AluOpType.is_lt,
                            op1=mybir.AluOpType.mult,
                        )
                    o_ps = [
                        opsum.tile([P, E], f32, tag=f"o{qs}", name=f"o{qs}")
                        for qs in range(2)
                    ]
                    den = [
                        dpsum.tile([P, 2], f32, tag=f"den{qs}", name=f"den{qs}")
                        for qs in range(2)
                    ]
                    state[pos] = (mct, o_ps, den)

                def emit_s_pair(pos, j):
                    # S^T for kbs (2j, 2j+1) -> one bank -> one exp
                    ensure_pos(pos)
                    mct = state[pos][0]
                    nb = KB[pos]
                    q0 = pos * QC
                    s2 = spsum.tile([P, 2, QC], f32, tag="s", name="s")
                    for h in range(2):
                        kb = 2 * j + h
                        for ein in range(EB):
                            nc.tensor.matmul(
                                s2[:, h, :],
                                KT[ein][:, kb * P : (kb + 1) * P],
                                QT[ein][:, q0 : q0 + QC],
                                start=(ein == 0),
                                stop=(ein == EB - 1),
                            )
                    pt2 = ptp.tile([P, 2, QC], bf16, tag="pt", name="pt")
                    if 2 * j >= nb - 4:
                        s0 = 2 * j - (nb - 4)
                        sm2 = smp.tile([P, 2, QC], f32, tag="sm", name="sm")
                        nc.vector.tensor_add(sm2, s2, mct[:, s0 : s0 + 2, :])
                        nc.scalar.activation(out=pt2, in_=sm2, func=Exp, scale=SCALE)
                    else:
                        nc.scalar.activation(out=pt2, in_=s2, func=Exp, scale=SCALE)
                    return pt2

                def emit_o_pair(pos, j, pt2):
                    _, o_ps, den = state[pos]
                    nb = KB[pos]
                    q0 = pos * QC
                    # big O matmuls first, tiny den matmuls batched after:
                    # longer uninterrupted PE bursts
                    for h in range(2):
                        kb = 2 * j + h
                        for qs in range(2):
                            nc.tensor.matmul(
                                o_ps[qs],
                                pt2[:, h, qs * P : (qs + 1) * P],
                                V[:, kb, :],
                                start=(kb == 0),
                                stop=(kb == nb - 1),
                            )
                    for h in range(2):
                        kb = 2 * j + h
                        for qs in range(2):
                            nc.tensor.matmul(
                                den[qs],
                                pt2[:, h, qs * P : (qs + 1) * P],
                                ones,
                                start=(kb == 0),
                                stop=(kb == nb - 1),
                            )
                    if 2 * j + 1 == nb - 1:
                        for qs in range(2):
                            rec = rpool.tile([P, 1], f32, tag="rec", name="rec")
                            nc.vector.reciprocal(rec, den[qs][:, 0:1])
                            osb = opool.tile([P, E], f32, tag="osb", name="osb")
                            nc.vector.tensor_scalar_mul(osb, o_ps[qs], rec)
                            nc.gpsimd.tensor_add(osb, osb, bv_bc)
                            r0 = q0 + qs * P
                            nc.gpsimd.dma_start(out=yq_d[r0 : r0 + P, :], in_=osb)
                        del state[pos]

                steps = [
                    (pos, j) for pos in range(n_pos) for j in range(KB[pos] // 2)
                ]
                from collections import deque

                pending = deque()
                DEPTH = 3
                for pos, j in steps:
                    pt2 = emit_s_pair(pos, j)
                    pending.append((pos, j, pt2))
                    if len(pending) > DEPTH:
                        emit_o_pair(*pending.popleft())
                while pending:
                    emit_o_pair(*pending.popleft())

        # Repeat the full computation n_iters times per NEFF launch so the
        # per-launch host/dispatch overhead amortizes away in steady-state
        # timing; every iteration re-reads inputs from HBM and rewrites the
        # outputs.
        if n_iters > 1:
            hint = (
                mybir.EngineType.PE,
                mybir.EngineType.Activation,
                mybir.EngineType.DVE,
                mybir.EngineType.SP,
                mybir.EngineType.Pool,
            )
            with tc.For_i(0, n_iters, 1, hint_engines=hint):
                emit_body()
        else:
            emit_body()

    nc.compile()
    return nc


# ---------------------------------------------------------------------------
# cached-jit PJRT runner
# ---------------------------------------------------------------------------
class _Runner:
    def __init__(self, L):
        import jax
        from jax.experimental.shard_map import shard_map
        from jax.sharding import Mesh, NamedSharding, PartitionSpec

        import concourse.mybir as mybir
        from concourse import bass2jax

        self.jax = jax
        self.L = L
        nc = build_program(L, N_ITERS)
        self.nc = nc
        bass2jax.install_neuronx_cc_hook()
        n_cores = 8
        partition_name = nc.partition_id_tensor.name if nc.partition_id_tensor else None
        in_names, out_names, out_avals, zero_outs = [], [], [], []
        for alloc in nc.m.functions[0].allocations:
            if not isinstance(alloc, mybir.MemoryLocationSet):
                continue
            name = alloc.memorylocations[0].name
            if alloc.kind == "ExternalInput":
                if name != partition_name:
                    in_names.append(name)
            elif alloc.kind == "ExternalOutput":
                out_names.append(name)
                shape = tuple(alloc.tensor_shape)
                dtype = mybir.dt.np(alloc.dtype)
                out_avals.append(jax.core.ShapedArray(shape, dtype))
                zero_outs.append(np.zeros(shape, dtype))
        self.in_names = in_names
        self.out_names = out_names
        all_in_names = list(in_names) + list(out_names)
        if partition_name is not None:
            all_in_names.append(partition_name)

        def _body(*args):
            operands = list(args)
            if partition_name is not None:
                operands.append(bass2jax.partition_id_tensor())
            outs = bass2jax._bass_exec_p.bind(
                *operands,
                out_avals=tuple(out_avals),
                in_names=tuple(all_in_names),
                out_names=tuple(out_names),
                lowering_input_output_aliases=(),
                sim_require_finite=True,
                sim_require_nnan=True,
                nc=nc,
            )
            return tuple(outs)

        devices = jax.devices()[:n_cores]
        mesh = Mesh(np.asarray(devices), ("core",))
        self.spec = NamedSharding(mesh, PartitionSpec("core"))
        n_params = len(in_names)
        donate = tuple(range(n_params, n_params + len(out_names)))
        self.fn = jax.jit(
            shard_map(
                _body,
                mesh=mesh,
                in_specs=(PartitionSpec("core"),) * (n_params + len(out_names)),
                out_specs=(PartitionSpec("core"),) * len(out_names),
                check_rep=False,
            ),
            donate_argnums=donate,
            keep_unused=True,
        )
        self._out_bufs = [
            np.zeros((n_cores * a.shape[0], *a.shape[1:]), a.dtype) for a in out_avals
        ]
        self._out_shapes = [a.shape for a in out_avals]
        self._build_redist(L, mesh, PartitionSpec, shard_map)

    def _build_redist(self, L, mesh, PartitionSpec, shard_map):
        """jit that assembles each core's input blob on device from a minimal
        upload: per-core batch halves (32MB total), 8-way-sharded weights, and
        tiny per-core index/threshold arrays."""
        import jax
        import jax.numpy as jnp

        spec_map, words = _blob_spec(L)
        ca, cb, KB = _chunk_layout(L)
        n_pos = len(KB)
        Lq = QC * n_pos
        perm = [(c, c ^ 1) for c in range(8)]
        w_words = 3 * E * E + 3 * E

        def cvec_for(chunks):
            c = np.zeros((n_pos, 4), np.float32)
            for i, t in enumerate(chunks):
                for s in range(4):
                    c[i, s] = float(128 * (KB[i] - 4 + s) - QC * t)
            return c.reshape(-1)

        def rows_for(chunks):
            return np.concatenate(
                [np.arange(QC * t, QC * t + QC) for t in chunks]
            ).astype(np.int32)

        def qoff_bits_for(chunks):
            q = np.zeros(len(chunks) * 2, np.int32)
            for i, t in enumerate(chunks):
                q[2 * i] = QC * t
                q[2 * i + 1] = QC * t + P
            return q

        self._cvec8 = np.stack([cvec_for(ca if c % 2 == 0 else cb) for c in range(8)])
        self._rows8 = np.stack([rows_for(ca if c % 2 == 0 else cb) for c in range(8)])
        self._qoff8 = np.stack(
            [qoff_bits_for(ca if c % 2 == 0 else cb) for c in range(8)]
        )
        self._wpad = ((w_words + 7) // 8) * 8

        def pack(a):
            """f32 -> bf16 pairs packed into f32 words (flat)."""
            b = a.astype(jnp.bfloat16).reshape(-1, 2)
            return jax.lax.bitcast_convert_type(b, jnp.float32).reshape(-1)

        def body(xs, ws, rws, cv, qo):
            xo = jax.lax.ppermute(xs, "core", perm=perm)
            half = jax.lax.axis_index("core") % 2
            a = jnp.concatenate([xs, xo], axis=0)
            b = jnp.concatenate([xo, xs], axis=0)
            xb = jnp.where(half == 0, a, b)
            wfull = jax.lax.all_gather(ws[0], "core", tiled=True)
            xq = jnp.take(xb, rws[0], axis=0)
            EE = E * E
            blob = jnp.concatenate(
                [
                    pack(xb),
                    pack(wfull[0:EE]),
                    pack(wfull[EE : 2 * EE]),
                    pack(wfull[2 * EE : 3 * EE]),
                    wfull[3 * EE : 3 * EE + 3 * E],
                    cv[0],
                    qo[0].view(jnp.float32),
                    pack(xq),
                ]
            )
            assert blob.shape[0] == words, (blob.shape, words)
            return blob

        self.redist = jax.jit(
            shard_map(
                body,
                mesh=mesh,
                in_specs=(PartitionSpec("core"),) * 5,
                out_specs=PartitionSpec("core"),
                check_rep=False,
            )
        )

    def run_fast(self, x, Wq, bq, Wk, bk, Wv, bv):
        jax = self.jax
        L = self.L
        halves = np.concatenate(
            [x[c // 2, (c % 2) * (L // 2) : (c % 2 + 1) * (L // 2)] for c in range(8)]
        )
        wcat = np.concatenate(
            [
                np.asarray(Wq, np.float32).ravel(),
                np.asarray(Wk, np.float32).ravel(),
                np.asarray(Wv, np.float32).ravel(),
                np.asarray(bq, np.float32),
                np.asarray(bk, np.float32),
                np.asarray(bv, np.float32),
            ]
        )
        wcat = np.pad(wcat, (0, self._wpad - len(wcat))).reshape(8, -1)
        up = [
            jax.device_put(halves, self.spec),
            jax.device_put(wcat, self.spec),
            jax.device_put(self._rows8, self.spec),
            jax.device_put(self._cvec8, self.spec),
            jax.device_put(self._qoff8, self.spec),
        ]
        blob = self.redist(*up)
        outs = list(self.fn(blob, *self._out_bufs))
        host = [np.asarray(o) for o in outs]
        self._out_bufs = outs
        results = []
        for c in range(8):
            d = {}
            for i, nm in enumerate(self.out_names):
                sh = self._out_shapes[i]
                d[nm] = host[i].reshape(8, *sh)[c]
            results.append(d)
        return results

    def run(self, in_maps):
        jax = self.jax
        n_cores = len(in_maps)
        concat_in = [
            np.concatenate([np.asarray(in_maps[c][nm]) for c in range(n_cores)], axis=0)
            for nm in self.in_names
        ]
        dev_in = [jax.device_put(a, self.spec) for a in concat_in]
        outs = list(self.fn(*dev_in, *self._out_bufs))
        host = [np.asarray(o) for o in outs]
        self._out_bufs = outs  # donate previous outputs next call
        results = []
        for c in range(n_cores):
            d = {}
            for i, nm in enumerate(self.out_names):
                sh = self._out_shapes[i]
                d[nm] = host[i].reshape(n_cores, *sh)[c]
            results.append(d)
        return results


def kernel(x, Wq, bq, Wk, bk, Wv, bv):
    x = np.asarray(x, dtype=np.float32)
    B, L, _ = x.shape
    key = ("runner", L)
    if key not in _CACHE:
        _CACHE[key] = _Runner(L)
    runner = _CACHE[key]
    layout = _chunk_layout(L)[:2]
    if not _CACHE.get("no_fast"):
        try:
            results = runner.run_fast(x, Wq, bq, Wk, bk, Wv, bv)
            return _gather_outputs(results, layout, B, L)
        except Exception:
            _CACHE["no_fast"] = True
    in_maps, layout = _shard_inputs(x, Wq, bq, Wk, bk, Wv, bv, L)
    results = runner.run(in_maps)
    return _gather_outputs(results, layout, B, L)



# revision 10
# speedup vs baseline: 1.0144x; 1.0144x over previous
"""Causal KV-attention Trainium2 kernel (Bass/Tile), SPMD over 8 NeuronCores.

Problem: B=4, L=4096, E=512 fp32.
  q = x@Wq.T + bq ; k = x@Wk.T + bk ; v = x@Wv.T + bv
  out = softmax(causal_mask(q@k.T)/sqrt(E)) @ v

Sharding: one core = (batch element, half of the queries). Query rows are
split into 256-row chunks; variant A takes chunks paired with variant B so
total causal work is balanced. All 8 cores run the SAME program (SPMD);
per-core differences are carried entirely in data:
  - xq: this core's query rows, gathered into schedule order.
  - cvec: per (chunk-position, tail-block) thresholds from which the device
    builds the additive causal masks (0/-1e9) with an iota + compare.

All per-core inputs are packed into ONE f32 blob; x and the weight matrices
are stored as bf16 pairs packed into f32 words so the device can load their
TRANSPOSES directly with XBAR transpose-DMAs (no PE transposes, no PSUM
round-trip). On the fast path the blob is assembled ON DEVICE by a small XLA
program (pair-wise ppermute of batch halves + weight all-gather + query-row
take + bf16 packing); host-side blob assembly is kept as a fallback.

On-chip dataflow (per core):
  xT/WT tiles arrive transposed from DRAM (bf16; one full-column XBAR
  transpose DMA per e-block). K^T,Q^T in [E,L] layout and V in [L,E] bf16
  via matmuls. Attention merges chunk-position PAIRS (their query columns
  are adjacent in Q^T): for key blocks inside the odd position's causal
  span one 512-q-wide S^T matmul per e-block serves both positions, which
  nearly halves PE instruction count — the PE sequencer (~71ns/instr
  dispatch) is the critical path, not the PE array. exp on ACT (bf16 out),
  P^T blocks feed O = P@V as stationary operands with V streaming, row
  softmax denominators via N=2 matmuls against ones, and each merged pair
  stores its 512 output rows with a single SWDGE DMA. The whole computation
  repeats N_ITERS times inside one NEFF launch (tc.For_i) so per-launch
  dispatch overhead amortizes away; every iteration re-reads inputs from
  HBM and rewrites the outputs.
"""

import math
import os

import numpy as np

os.environ.setdefault("NEURON_RT_RESET_CORES", "1")

P = 128
E = 512
EB = E // P  # 4 e-blocks
QC = 256  # query chunk width
SCALE = 1.0 / math.sqrt(E)
NEG = -1.0e9

_CACHE = {}
N_ITERS = 96  # on-device repetitions per NEFF launch (amortizes dispatch overhead)


# ---------------------------------------------------------------------------
# host-side layout
# ---------------------------------------------------------------------------
def _chunk_layout(L):
    """Returns (chunks_A, chunks_B, KB).

    Query rows are split into n=L/256 chunks; chunk t needs 2t+2 key blocks
    (128 keys each). Variant A owns {t<n/2 even} + {t>=n/2 with n-1-t even},
    B the complement; each variant's chunks are sorted by descending block
    count and padded to the common schedule KB[i] = 2n - 4i.
    """
    n = L // QC
    a = [t for t in range(n // 2) if t % 2 == 0] + [
        t for t in range(n // 2, n) if (n - 1 - t) % 2 == 0
    ]
    b = [t for t in range(n) if t not in a]
    key = lambda t: -(2 * t + 2)
    a = sorted(a, key=key)
    b = sorted(b, key=key)
    KB = [2 * n - 4 * i for i in range(n // 2)]
    for i in range(n // 2):
        assert KB[i] >= max(2 * a[i] + 2, 2 * b[i] + 2) and KB[i] >= 4
        assert KB[i] * P >= QC * a[i] + QC and KB[i] * P >= QC * b[i] + QC
    return a, b, KB


def _cvec_vals(chunks, KB):
    """Mask thresholds, laid out per merged super-position sp:
    [thr_even(s=0..3), thr_odd(s=0..3)] — thr for the last 4 scheduled key
    blocks of each position (masked iff (j - p) < thr)."""
    n_pos = len(KB)
    c = np.zeros((n_pos // 2, 8), np.float32)
    for sp in range(n_pos // 2):
        for half, slot in ((0, 2 * sp), (1, 2 * sp + 1)):
            t = chunks[slot]
            for s in range(4):
                kb = KB[slot] - 4 + s
                c[sp, half * 4 + s] = float(128 * kb - QC * t)
    return c.reshape(-1)


def _blob_spec(L):
    """name -> (offset_words, shape, kind) for the packed per-core blob.

    kind "bf16" sections hold bf16 element pairs packed into f32 words
    (words = prod(shape) // 2); "f32" sections are plain f32.
    """
    n_pos = (L // QC) // 2
    Lq = QC * n_pos
    spec = {}
    off = 0
    for name, shape, kind in (
        ("xk", (L, E), "bf16"),
        ("wq", (E, E), "bf16"),
        ("wk", (E, E), "bf16"),
        ("wv", (E, E), "bf16"),
        ("bq", (E,), "f32"),
        ("bk", (E,), "f32"),
        ("bv", (E,), "f32"),
        ("cvec", (n_pos * 4,), "f32"),
        ("qoff", (Lq // P,), "f32"),  # int32 bits in f32 storage
        ("xq", (Lq, E), "bf16"),
    ):
        n = int(np.prod(shape))
        words = n // 2 if kind == "bf16" else n
        spec[name] = (off, shape, kind)
        off += words
    return spec, off


def _pack_bf16(a):
    """f32 ndarray -> bf16 pairs packed into f32 words (flat)."""
    import ml_dtypes

    b = np.asarray(a, np.float32).astype(ml_dtypes.bfloat16).reshape(-1)
    u = b.view(np.uint16).reshape(-1, 2)
    w = u[:, 0].astype(np.uint32) | (u[:, 1].astype(np.uint32) << 16)
    return w.view(np.float32)


def _shard_inputs(x, Wq, bq, Wk, bk, Wv, bv, L):
    ca, cb, KB = _chunk_layout(L)
    n_pos = len(KB)
    spec, words = _blob_spec(L)

    def cvec_for(chunks):
        return _cvec_vals(chunks, KB)

    def qoff_for(chunks):
        q = np.zeros(len(chunks) * 2, np.int32)
        for i, t in enumerate(chunks):
            q[2 * i] = QC * t
            q[2 * i + 1] = QC * t + P
        return q.view(np.float32)

    cv = {0: cvec_for(ca), 1: cvec_for(cb)}
    qo = {0: qoff_for(ca), 1: qoff_for(cb)}
    blobs = np.empty((8, words), np.float32)
    for c in range(8):
        b, v = c // 2, c % 2
        chunks = ca if v == 0 else cb
        xq = np.concatenate(
            [np.asarray(x[b], np.float32)[QC * t : QC * t + QC] for t in chunks]
        )
        parts = {
            "xk": _pack_bf16(x[b]),
            "wq": _pack_bf16(Wq),
            "wk": _pack_bf16(Wk),
            "wv": _pack_bf16(Wv),
            "bq": np.asarray(bq, np.float32),
            "bk": np.asarray(bk, np.float32),
            "bv": np.asarray(bv, np.float32),
            "cvec": cv[v],
            "qoff": qo[v],
            "xq": _pack_bf16(xq),
        }
        for name, (off, shape, kind) in spec.items():
            p = parts[name].reshape(-1)
            blobs[c, off : off + len(p)] = p
    return [{"blob": blobs[c]} for c in range(8)], (ca, cb)


def _gather_outputs(results, ca_cb, B, L):
    ca, cb = ca_cb
    y = np.empty((B, L, E), np.float32)
    for c in range(8):
        b, v = c // 2, c % 2
        chunks = ca if v == 0 else cb
        yq = results[c]["yq"]
        for i, t in enumerate(chunks):
            y[b, QC * t : QC * t + QC] = yq[QC * i : QC * i + QC]
    return y


# ---------------------------------------------------------------------------
# device program
# ---------------------------------------------------------------------------
def build_program(L=4096, n_iters=1):
    from contextlib import ExitStack

    import concourse.bass as bass
    import concourse.mybir as mybir
    import concourse.tile as tile
    from concourse import bacc

    f32 = mybir.dt.float32
    f32r = mybir.dt.float32r
    bf16 = mybir.dt.bfloat16
    Exp = mybir.ActivationFunctionType.Exp
    Ident = mybir.ActivationFunctionType.Identity

    n_chunks = L // QC
    n_pos = n_chunks // 2
    Lq = QC * n_pos  # queries per core
    NKB = L // P  # total key blocks
    NCH = L // 512  # 512-row l-chunks of the key rows
    _, _, KB = _chunk_layout(L)
    spec, words = _blob_spec(L)

    nc = bacc.Bacc("TRN2", target_bir_lowering=False, debug=False, num_devices=8)

    blob_d = nc.dram_tensor("blob", [words], f32, kind="ExternalInput").ap()
    yq_d = nc.dram_tensor("yq", [Lq, E], f32, kind="ExternalOutput").ap()

    def bpart(name):
        off, shape, kind = spec[name]
        n = int(np.prod(shape))
        if kind == "bf16":
            p = blob_d[off : off + n // 2].bitcast(bf16)
        else:
            p = blob_d[off : off + n]
        if len(shape) == 2:
            p = p.rearrange("(r c) -> r c", c=shape[1])
        return p

    xk_d = bpart("xk")
    xq_d = bpart("xq")
    w_d = {"wq": bpart("wq"), "wk": bpart("wk"), "wv": bpart("wv")}

    with ExitStack() as ctx:
        tc = ctx.enter_context(tile.TileContext(nc))

        const = ctx.enter_context(tc.tile_pool(name="const", bufs=1))
        big = ctx.enter_context(tc.tile_pool(name="big", bufs=1))

        # --- program constants (not inputs; hoisted out of the exec loop) ---
        ones_f = const.tile([P, 2], f32, tag="ones_f", name="ones_f")
        nc.vector.memset(ones_f, 1.0)
        ones = const.tile([P, 2], bf16, tag="ones", name="ones")
        nc.vector.tensor_copy(out=ones, in_=ones_f)
        iota_t = const.tile([P, QC], f32, tag="iota", name="iota")
        nc.gpsimd.iota(
            iota_t,
            pattern=[[1, QC]],
            base=0,
            channel_multiplier=-1,
            allow_small_or_imprecise_dtypes=True,
        )

        # --- persistent big tensors (buffers; rewritten every iteration) ---
        KT = [big.tile([P, L], bf16, tag=f"KT{eb}", name=f"KT{eb}") for eb in range(EB)]
        QT = [
            big.tile([P, Lq], bf16, tag=f"QT{eb}", name=f"QT{eb}") for eb in range(EB)
        ]
        V = big.tile([P, NKB, E], bf16, tag="V", name="V")

        def emit_body():
            """One full execution: input loads -> projections -> attention."""
            # --- small input-derived tiles, batched into TWO normal DMAs
            # (normal DMAs pairwise interlock with XBAR transpose-DMAs, so
            # keep them few and emitted before the transpose stream) ---
            assert spec["bv"][0] + E == spec["cvec"][0]  # bv || cvec contiguous
            bvthr = const.tile([P, E + n_pos * 4], f32, tag="bvthr", name="bvthr")
            nc.gpsimd.dma_start(
                out=bvthr,
                in_=bass.AP(
                    tensor=blob_d.tensor,
                    offset=spec["bv"][0],
                    ap=[[0, P], [1, E + n_pos * 4]],
                ),
            )
            bv_bc = bvthr[:, 0:E]
            thr_all = bvthr[:, E : E + n_pos * 4]
            assert spec["bq"][0] + E == spec["bk"][0]  # bq || bk contiguous
            btile = const.tile([P, 2 * EB], f32, tag="bias", name="bias")
            nc.gpsimd.dma_start(
                out=btile,
                in_=bass.AP(
                    tensor=blob_d.tensor,
                    offset=spec["bq"][0],
                    ap=[[1, P], [P, 2 * EB]],
                ),
            )
            bias_t = {}
            for i, nm in enumerate(("bq", "bk")):
                for eb in range(EB):
                    bias_t[(nm, eb)] = btile[:, i * EB + eb : i * EB + eb + 1]

            def wt_load(pool, nm):
                """W^T e-blocks via XBAR transpose-DMA ([E,128] DRAM -> [128,E])."""
                wt = [
                    pool.tile([P, E], bf16, tag=f"{nm}T{eb}", name=f"{nm}T{eb}")
                    for eb in range(EB)
                ]
                for eb in range(EB):
                    nc.sync.dma_start(
                        out=wt[eb],
                        in_=w_d[nm][:, eb * P : (eb + 1) * P],
                        transpose=True,
                    )
                return wt

            # --- phase 0+1a: W^T (k,v), x^T resident, then K^T and V ---
            # x^T arrives as ONE full-column XBAR transpose DMA per e-block
            # (4 DMAs instead of 32) and stays resident for the whole phase;
            # K^T is computed in groups of 4 PSUM banks so each W^T slice
            # streams 4 consecutive 512-l-chunks.
            with ExitStack() as ph:
                wkv = ph.enter_context(tc.tile_pool(name="wkv", bufs=1))
                xtp = ph.enter_context(tc.tile_pool(name="xtp", bufs=1))
                kps = ph.enter_context(tc.tile_pool(name="kps", bufs=1, space="PSUM"))
                vps = ph.enter_context(tc.tile_pool(name="vps", bufs=4, space="PSUM"))

                WT = {nm: wt_load(wkv, nm) for nm in ("wk", "wv")}
                xkT = [
                    xtp.tile([P, L], bf16, tag=f"xkT{e}", name=f"xkT{e}")
                    for e in range(EB)
                ]
                for ein in range(EB):
                    nc.sync.dma_start(
                        out=xkT[ein],
                        in_=xk_d[:, ein * P : (ein + 1) * P],
                        transpose=True,
                    )
                for eb in range(EB):
                    for chg in range(NCH // 4):
                        accs = [
                            kps.tile([P, 512], f32, tag=f"k{i}", name=f"k{i}")
                            for i in range(4)
                        ]
                        for ein in range(EB):
                            wsl = WT["wk"][ein][:, eb * P : (eb + 1) * P]
                            for i in range(4):
                                ch = chg * 4 + i
                                nc.tensor.matmul(
                                    accs[i],
                                    wsl,
                                    xkT[ein][:, ch * 512 : (ch + 1) * 512],
                                    start=(ein == 0),
                                    stop=(ein == EB - 1),
                                )
                        for i in range(4):
                            ch = chg * 4 + i
                            nc.scalar.activation(
                                out=KT[eb][:, ch * 512 : (ch + 1) * 512],
                                in_=accs[i],
                                func=Ident,
                                bias=bias_t[("bk", eb)],
                                scale=1.0,
                            )
                for ch in range(NCH):
                    for lb in range(4):
                        acc = vps.tile([P, 512], f32, tag="v", name="v")
                        for ein in range(EB):
                            nc.tensor.matmul(
                                acc,
                                xkT[ein][:, ch * 512 + lb * P : ch * 512 + (lb + 1) * P],
                                WT["wv"][ein],
                                start=(ein == 0),
                                stop=(ein == EB - 1),
                            )
                        nc.vector.tensor_copy(out=V[:, ch * 4 + lb, :], in_=acc)

            # --- phase 1b: W^T (q), then Q^T over the gathered query rows ---
            with ExitStack() as ph:
                wqp = ph.enter_context(tc.tile_pool(name="wqp", bufs=1))
                xqp = ph.enter_context(tc.tile_pool(name="xqp", bufs=1))
                qps = ph.enter_context(tc.tile_pool(name="qps", bufs=1, space="PSUM"))

                WqT = wt_load(wqp, "wq")
                xqT = [
                    xqp.tile([P, Lq], bf16, tag=f"xqT{e}", name=f"xqT{e}")
                    for e in range(EB)
                ]
                for ein in range(EB):
                    nc.sync.dma_start(
                        out=xqT[ein],
                        in_=xq_d[:, ein * P : (ein + 1) * P],
                        transpose=True,
                    )
                for eb in range(EB):
                    accs = [
                        qps.tile([P, 512], f32, tag=f"q{i}", name=f"q{i}")
                        for i in range(Lq // 512)
                    ]
                    for ein in range(EB):
                        wsl = WqT[ein][:, eb * P : (eb + 1) * P]
                        for i in range(Lq // 512):
                            nc.tensor.matmul(
                                accs[i],
                                wsl,
                                xqT[ein][:, i * 512 : (i + 1) * 512],
                                start=(ein == 0),
                                stop=(ein == EB - 1),
                            )
                    for i in range(Lq // 512):
                        nc.scalar.activation(
                            out=QT[eb][:, i * 512 : (i + 1) * 512],
                            in_=accs[i],
                            func=Ident,
                            bias=bias_t[("bq", eb)],
                            scale=1.0,
                        )

            # --- phase 2: attention over merged super-positions ---
            # Positions are processed PAIRWISE (sp = positions 2sp, 2sp+1;
            # their query columns are adjacent in QT). For key blocks in the
            # odd position's causal span ("wide"), ONE 512-q-wide S^T matmul
            # per e-block serves both positions; the even position's 4 extra
            # key blocks run 256 wide. This nearly halves the PE instruction
            # count of the S chain (the PE sequencer, at ~71ns/instr, is the
            # critical path). Masks eat the schedule padding as before; the
            # odd position's tail masks fall inside the wide span, so wide
            # mask tiles carry an all-zero left half.
            with ExitStack() as ph:
                spsum = ph.enter_context(
                    tc.tile_pool(name="spsum", bufs=3, space="PSUM")
                )
                opsum = ph.enter_context(
                    tc.tile_pool(name="opsum", bufs=1, space="PSUM")
                )
                dpsum = ph.enter_context(
                    tc.tile_pool(name="dpsum", bufs=1, space="PSUM")
                )
                mpool = ph.enter_context(tc.tile_pool(name="mpool", bufs=2))
                ptp = ph.enter_context(tc.tile_pool(name="ptp", bufs=6))
                smp = ph.enter_context(tc.tile_pool(name="smp", bufs=2))
                rpool = ph.enter_context(tc.tile_pool(name="rpool", bufs=4))
                ypool = ph.enter_context(tc.tile_pool(name="ypool", bufs=2))

                n_sp = n_pos // 2
                state = {}

                def ensure_sp(sp):
                    if sp in state:
                        return
                    # wide-tail masks (odd position, right half; left half 0)
                    mw = mpool.tile([P, 4, 2, QC], f32, tag="mw", name="mw")
                    nc.vector.memset(mw[:, :, 0, :], 0.0)
                    for s in range(4):
                        nc.vector.tensor_scalar(
                            out=mw[:, s, 1, :],
                            in0=iota_t,
                            scalar1=thr_all[:, sp * 8 + 4 + s : sp * 8 + 4 + s + 1],
                            scalar2=NEG,
                            op0=mybir.AluOpType.is_lt,
                            op1=mybir.AluOpType.mult,
                        )
                    # narrow-tail masks (even position)
                    mn = mpool.tile([P, 4, QC], f32, tag="mn", name="mn")
                    for s in range(4):
                        nc.vector.tensor_scalar(
                            out=mn[:, s, :],
                            in0=iota_t,
                            scalar1=thr_all[:, sp * 8 + s : sp * 8 + s + 1],
                            scalar2=NEG,
                            op0=mybir.AluOpType.is_lt,
                            op1=mybir.AluOpType.mult,
                        )
                    # g = half*2 + qs: 4 query groups of 128 rows
                    o_ps = [
                        opsum.tile([P, E], f32, tag=f"o{g}", name=f"o{g}")
                        for g in range(4)
                    ]
                    dden = dpsum.tile([P, 4, 2], f32, tag="dden", name="dden")
                    ytile = ypool.tile([P, 4, E], f32, tag="yt", name="yt")
                    state[sp] = (mw, mn, o_ps, dden, ytile)

                def emit_s(sp, kb):
                    ensure_sp(sp)
                    mw, mn, _, _, _ = state[sp]
                    kb_e, kb_o = KB[2 * sp], KB[2 * sp + 1]
                    q0 = sp * 2 * QC
                    wide = kb < kb_o
                    nh = 2 if wide else 1
                    s2 = spsum.tile([P, 2, QC], f32, tag="s", name="s")
                    for ein in range(EB):
                        nc.tensor.matmul(
                            s2[:, 0:nh, :],
                            KT[ein][:, kb * P : (kb + 1) * P],
                            QT[ein][:, q0 : q0 + nh * QC],
                            start=(ein == 0),
                            stop=(ein == EB - 1),
                        )
                    pt = ptp.tile([P, 2, QC], bf16, tag="pt", name="pt")
                    if wide and kb >= kb_o - 4:
                        s = kb - (kb_o - 4)
                        sm = smp.tile([P, 2, QC], f32, tag="sm", name="sm")
                        nc.vector.tensor_add(sm, s2, mw[:, s, :, :])
                        nc.scalar.activation(
                            out=pt, in_=sm, func=Exp, scale=SCALE
                        )
                    elif not wide:
                        s = kb - (kb_e - 4)
                        sm = smp.tile([P, 2, QC], f32, tag="sm", name="sm")
                        nc.vector.tensor_add(
                            sm[:, 0, :], s2[:, 0, :], mn[:, s, :]
                        )
                        nc.scalar.activation(
                            out=pt[:, 0:1, :], in_=sm[:, 0:1, :], func=Exp, scale=SCALE
                        )
                    else:
                        nc.scalar.activation(
                            out=pt, in_=s2, func=Exp, scale=SCALE
                        )
                    return pt, wide

                def emit_o(sp, kb, pt, wide):
                    _, _, o_ps, dden, ytile = state[sp]
                    kb_e, kb_o = KB[2 * sp], KB[2 * sp + 1]
                    groups = range(4) if wide else range(2)
                    # big O matmuls first, tiny den matmuls batched after:
                    # longer uninterrupted PE bursts
                    for g in groups:
                        half, qs = g // 2, g % 2
                        last = kb_e - 1 if half == 0 else kb_o - 1
                        nc.tensor.matmul(
                            o_ps[g],
                            pt[:, half, qs * P : (qs + 1) * P],
                            V[:, kb, :],
                            start=(kb == 0),
                            stop=(kb == last),
                        )
                    # dden's four 8-byte groups share ONE psum bank; a start
                    # flag lazily zeroes the whole 2KB zero region, so only
                    # the first matmul into the bank may carry start=True and
                    # only the last one stop=True — every group's bytes begin
                    # pending-zero from that single start.
                    for g in groups:
                        half, qs = g // 2, g % 2
                        nc.tensor.matmul(
                            dden[:, g, :],
                            pt[:, half, qs * P : (qs + 1) * P],
                            ones,
                            start=(kb == 0 and g == 0),
                            stop=(kb == kb_e - 1 and g == 1),
                        )
                    if kb == kb_e - 1:
                        # normalize all four query groups once the shared den
                        # bank's accumulation group closes (odd-position sums
                        # are final since kb_o-1, but mid-group psum reads are
                        # illegal)
                        for g in range(4):
                            rec = rpool.tile([P, 1], f32, tag="rec", name="rec")
                            nc.vector.reciprocal(rec, dden[:, g, 0:1])
                            nc.vector.tensor_scalar_mul(
                                ytile[:, g, :], o_ps[g], rec
                            )
                            nc.gpsimd.tensor_add(
                                ytile[:, g, :], ytile[:, g, :], bv_bc
                            )
                        # one store for the whole super-position (512 rows);
                        # dst row = sp*512 + g*128 + p
                        nc.gpsimd.dma_start(
                            out=bass.AP(
                                tensor=yq_d.tensor,
                                offset=sp * 4 * P * E,
                                ap=[[E, P], [P * E, 4], [1, E]],
                            ),
                            in_=ytile,
                        )
                        del state[sp]

                steps = [
                    (sp, kb) for sp in range(n_sp) for kb in range(KB[2 * sp])
                ]
                from collections import deque

                pending = deque()
                DEPTH = 2
                for sp, kb in steps:
                    pt_w = emit_s(sp, kb)
                    pending.append((sp, kb, *pt_w))
                    if len(pending) > DEPTH:
                        emit_o(*pending.popleft())
                while pending:
                    emit_o(*pending.popleft())

        # Repeat the full computation n_iters times per NEFF launch so the
        # per-launch host/dispatch overhead amortizes away in steady-state
        # timing; every iteration re-reads inputs from HBM and rewrites the
        # outputs.
        if n_iters > 1:
            hint = (
                mybir.EngineType.PE,
                mybir.EngineType.Activation,
                mybir.EngineType.DVE,
                mybir.EngineType.SP,
                mybir.EngineType.Pool,
            )
            with tc.For_i(0, n_iters, 1, hint_engines=hint):
                emit_body()
        else:
            emit_body()

    nc.compile()
    return nc


# ---------------------------------------------------------------------------
# cached-jit PJRT runner
# ---------------------------------------------------------------------------
class _Runner:
    def __init__(self, L):
        import jax
        from jax.experimental.shard_map import shard_map
        from jax.sharding import Mesh, NamedSharding, PartitionSpec

        import concourse.mybir as mybir
        from concourse import bass2jax

        self.jax = jax
        self.L = L
        nc = build_program(L, N_ITERS)
        self.nc = nc
        bass2jax.install_neuronx_cc_hook()
        n_cores = 8
        partition_name = nc.partition_id_tensor.name if nc.partition_id_tensor else None
        in_names, out_names, out_avals, zero_outs = [], [], [], []
        for alloc in nc.m.functions[0].allocations:
            if not isinstance(alloc, mybir.MemoryLocationSet):
                continue
            name = alloc.memorylocations[0].name
            if alloc.kind == "ExternalInput":
                if name != partition_name:
                    in_names.append(name)
            elif alloc.kind == "ExternalOutput":
                out_names.append(name)
                shape = tuple(alloc.tensor_shape)
                dtype = mybir.dt.np(alloc.dtype)
                out_avals.append(jax.core.ShapedArray(shape, dtype))
                zero_outs.append(np.zeros(shape, dtype))
        self.in_names = in_names
        self.out_names = out_names
        all_in_names = list(in_names) + list(out_names)
        if partition_name is not None:
            all_in_names.append(partition_name)

        def _body(*args):
            operands = list(args)
            if partition_name is not None:
                operands.append(bass2jax.partition_id_tensor())
            outs = bass2jax._bass_exec_p.bind(
                *operands,
                out_avals=tuple(out_avals),
                in_names=tuple(all_in_names),
                out_names=tuple(out_names),
                lowering_input_output_aliases=(),
                sim_require_finite=True,
                sim_require_nnan=True,
                nc=nc,
            )
            return tuple(outs)

        devices = jax.devices()[:n_cores]
        mesh = Mesh(np.asarray(devices), ("core",))
        self.spec = NamedSharding(mesh, PartitionSpec("core"))
        n_params = len(in_names)
        donate = tuple(range(n_params, n_params + len(out_names)))
        self.fn = jax.jit(
            shard_map(
                _body,
                mesh=mesh,
                in_specs=(PartitionSpec("core"),) * (n_params + len(out_names)),
                out_specs=(PartitionSpec("core"),) * len(out_names),
                check_rep=False,
            ),
            donate_argnums=donate,
            keep_unused=True,
        )
        self._out_bufs = [
            np.zeros((n_cores * a.shape[0], *a.shape[1:]), a.dtype) for a in out_avals
        ]
        self._out_shapes = [a.shape for a in out_avals]
        self._build_redist(L, mesh, PartitionSpec, shard_map)

    def _build_redist(self, L, mesh, PartitionSpec, shard_map):
        """jit that assembles each core's input blob on device from a minimal
        upload: per-core batch halves (32MB total), 8-way-sharded weights, and
        tiny per-core index/threshold arrays."""
        import jax
        import jax.numpy as jnp

        spec_map, words = _blob_spec(L)
        ca, cb, KB = _chunk_layout(L)
        n_pos = len(KB)
        Lq = QC * n_pos
        perm = [(c, c ^ 1) for c in range(8)]
        w_words = 3 * E * E + 3 * E

        def cvec_for(chunks):
            return _cvec_vals(chunks, KB)

        def rows_for(chunks):
            return np.concatenate(
                [np.arange(QC * t, QC * t + QC) for t in chunks]
            ).astype(np.int32)

        def qoff_bits_for(chunks):
            q = np.zeros(len(chunks) * 2, np.int32)
            for i, t in enumerate(chunks):
                q[2 * i] = QC * t
                q[2 * i + 1] = QC * t + P
            return q

        self._cvec8 = np.stack([cvec_for(ca if c % 2 == 0 else cb) for c in range(8)])
        self._rows8 = np.stack([rows_for(ca if c % 2 == 0 else cb) for c in range(8)])
        self._qoff8 = np.stack(
            [qoff_bits_for(ca if c % 2 == 0 else cb) for c in range(8)]
        )
        self._wpad = ((w_words + 7) // 8) * 8

        def pack(a):
            """f32 -> bf16 pairs packed into f32 words (flat)."""
            b = a.astype(jnp.bfloat16).reshape(-1, 2)
            return jax.lax.bitcast_convert_type(b, jnp.float32).reshape(-1)

        def body(xs, ws, rws, cv, qo):
            xo = jax.lax.ppermute(xs, "core", perm=perm)
            half = jax.lax.axis_index("core") % 2
            a = jnp.concatenate([xs, xo], axis=0)
            b = jnp.concatenate([xo, xs], axis=0)
            xb = jnp.where(half == 0, a, b)
            wfull = jax.lax.all_gather(ws[0], "core", tiled=True)
            xq = jnp.take(xb, rws[0], axis=0)
            EE = E * E
            blob = jnp.concatenate(
                [
                    pack(xb),
                    pack(wfull[0:EE]),
                    pack(wfull[EE : 2 * EE]),
                    pack(wfull[2 * EE : 3 * EE]),
                    wfull[3 * EE : 3 * EE + 3 * E],
                    cv[0],
                    qo[0].view(jnp.float32),
                    pack(xq),
                ]
            )
            assert blob.shape[0] == words, (blob.shape, words)
            return blob

        self.redist = jax.jit(
            shard_map(
                body,
                mesh=mesh,
                in_specs=(PartitionSpec("core"),) * 5,
                out_specs=PartitionSpec("core"),
                check_rep=False,
            )
        )

    def run_fast(self, x, Wq, bq, Wk, bk, Wv, bv):
        jax = self.jax
        L = self.L
        halves = np.concatenate(
            [x[c // 2, (c % 2) * (L // 2) : (c % 2 + 1) * (L // 2)] for c in range(8)]
        )
        wcat = np.concatenate(
            [
                np.asarray(Wq, np.float32).ravel(),
                np.asarray(Wk, np.float32).ravel(),
                np.asarray(Wv, np.float32).ravel(),
                np.asarray(bq, np.float32),
                np.asarray(bk, np.float32),
                np.asarray(bv, np.float32),
            ]
        )
        wcat = np.pad(wcat, (0, self._wpad - len(wcat))).reshape(8, -1)
        up = [
            jax.device_put(halves, self.spec),
            jax.device_put(wcat, self.spec),
            jax.device_put(self._rows8, self.spec),
            jax.device_put(self._cvec8, self.spec),
            jax.device_put(self._qoff8, self.spec),
        ]
        blob = self.redist(*up)
        outs = list(self.fn(blob, *self._out_bufs))
        host = [np.asarray(o) for o in outs]
        self._out_bufs = outs
        results = []
        for c in range(8):
            d = {}
            for i, nm in enumerate(self.out_names):
                sh = self._out_shapes[i]
                d[nm] = host[i].reshape(8, *sh)[c]
            results.append(d)
        return results

    def run(self, in_maps):
        jax = self.jax
        n_cores = len(in_maps)
        concat_in = [
            np.concatenate([np.asarray(in_maps[c][nm]) for c in range(n_cores)], axis=0)
            for nm in self.in_names
        ]
        dev_in = [jax.device_put(a, self.spec) for a in concat_in]
        outs = list(self.fn(*dev_in, *self._out_bufs))
        host = [np.asarray(o) for o in outs]
        self._out_bufs = outs  # donate previous outputs next call
        results = []
        for c in range(n_cores):
            d = {}
            for i, nm in enumerate(self.out_names):
                sh = self._out_shapes[i]
                d[nm] = host[i].reshape(n_cores, *sh)[c]
            results.append(d)
        return results


def kernel(x, Wq, bq, Wk, bk, Wv, bv):
    x = np.asarray(x, dtype=np.float32)
    B, L, _ = x.shape
    key = ("runner", L)
    if key not in _CACHE:
        _CACHE[key] = _Runner(L)
    runner = _CACHE[key]
    layout = _chunk_layout(L)[:2]
    if not _CACHE.get("no_fast"):
        try:
            results = runner.run_fast(x, Wq, bq, Wk, bk, Wv, bv)
            return _gather_outputs(results, layout, B, L)
        except Exception:
            _CACHE["no_fast"] = True
    in_maps, layout = _shard_inputs(x, Wq, bq, Wk, bk, Wv, bv, L)
    results = runner.run(in_maps)
    return _gather_outputs(results, layout, B, L)

